# revision 85
# baseline (speedup 1.0000x reference)
"""v5: query+key compacted sparse causal attention.

Sharding: 8 cores = 4 batches x 2 key-parity shards (key blocks of 128
interleaved across the core pair); host sums the two partial outputs.

Work reduction: keys dead under v_mask and queries dead under q_mask are
both compacted away on host (order-preserving), roughly quartering the
dense work.  Causal masking survives compaction because the
column->original-row map is monotone: for key row r, mask(col c) =
(c >= tc[r]) with tc[r] = searchsorted(q_orig, k_orig[r]) host data.
Boundary blocks also skip their fully-masked leading columns (c0b
subranges, ISA-aligned to 32) in the score/exp/mask/pv ops.

Schedule (per core, Activation-saturated ~12.5us of a ~21us kernel):
- superblock order (1, 2, big..., 0): small sbs warm the pipeline, the
  smallest drains last so the serial pv->copy->DMA tail is short;
- kt/qt packed into one dram param in need order so the first compute
  chunk is a single DMA; iot/vp/mask tiles ride Pool->SWDGE in parallel
  with the SP->HWDGE chunks;
- dummy bf16 matmuls warm the PE p-state before real work arrives
  (memset on DVE so Pool can start descriptor-gen immediately);
- mid sbs mask via one DVE scalar_tensor_tensor per boundary block
  (p *= iota >= tc - col0); the last two sbs instead fold the mask into
  the score psum with an identity-stationary matmul adding -1e9 tiles
  (PE has slack there, DVE would serialize the drain tail);
- output drains: DVE copies mid-kernel, Activation for the last sb
  (idle after its final exp).

Softmax runs unshifted (exp, no max-sub, fp32 psum accumulate) with the
denominator as a 65th V' column, divided out on host; padding keys carry
V'=0 and padding query columns are dropped by the host scatter.  Rows
before the first live key fall back to a host fp64 softmax.
"""

import sys

import numpy as np

try:
    import concourse  # noqa: F401
except ImportError:  # pragma: no cover
    sys.path.insert(0, "/opt/trn_rl_repo")

B, T, D = 4, 4096, 64
NCORES = 8
KB = 128
GRP = 2
N_WARMUP = 5
NEG_BIG = 1e9
PAD_TH = 1.0e9  # th sentinel for padding key rows (masks them to 0)

_compiled = {}


def _sb_order(nsb):
    """Small ragged sbs first (cheap pipeline warm-up), big ones in the
    middle, smallest (sb0) last so the serial drain tail is short."""
    if nsb == 1:
        return (0,)
    return tuple(range(1, 3)[:nsb - 1]) + tuple(range(nsb - 1, 2, -1)) + (0,)


def _tail_maskadd_sbs(order):
    """Tail sbs whose causal masks fold into the score psum via an
    identity-stationary matmul (PE) instead of a DVE multiply — takes the
    serial mask stage out of the drain tail.  Only safe for late sbs
    whose mask tiles have plenty of time to arrive."""
    return set(order[-2:]) if len(order) >= 4 else set()


def _qk_layout(widths, blocks, nslots, c0bs):
    """Need-ordered packing of kt|qt into one dram param so each input
    chunk is ONE contiguous DMA.  Returns (chunks, k_off, q_off, total)
    where chunks is a list of (start, end) col ranges in emission order,
    k_off[kb] is the param col of key block kb, q_off[sb] of sb's col0."""
    nsb = len(widths)
    order = _sb_order(nsb)
    col0s = np.concatenate([[0], np.cumsum(widths)]).astype(int)
    qpad = int(col0s[-1])
    ktw = blocks[-1] * KB
    c_a = int(col0s[order[0]])
    c_m = int(col0s[order[0] + 1])
    c_b = int(col0s[order[1] + 1]) if nsb > 1 else qpad
    kh0 = blocks[order[0]] * KB
    kh = blocks[order[min(1, nsb - 1)]] * KB

    pieces = [("k", 0, kh0), ("q", c_a, c_m)]
    if kh0 < kh:
        pieces.append(("k", kh0, kh))
    if c_m < c_b:
        pieces.append(("q", c_m, c_b))
    if c_b < qpad:
        pieces.append(("q", c_b, qpad))
    if kh < ktw:
        pieces.append(("k", kh, ktw))
    if c_a > 0:
        pieces.append(("q", 0, c_a))
    # chunk boundaries: first two pieces are chunk 0 (the critical first
    # transfer); after that one chunk per piece
    k_off = {}
    q_off = {}
    bounds = [0]
    off = 0
    for i, (kind, lo, hi) in enumerate(pieces):
        if kind == "k":
            for kb in range(lo // KB, hi // KB):
                k_off[kb] = off + (kb * KB - lo)
        else:
            for sb in range(nsb):
                if lo <= col0s[sb] and col0s[sb + 1] <= hi:
                    q_off[sb] = off + (int(col0s[sb]) - lo)
        off += hi - lo
        if i != 0:
            bounds.append(off)
    chunks = list(zip(bounds, bounds[1:]))
    return chunks, k_off, q_off, off, pieces


def _build_nc(widths, blocks, nslots, c0bs):
    import concourse.bass as bass
    import concourse.mybir as mybir
    import concourse.tile as tile
    from concourse import bacc

    f32 = mybir.dt.float32
    f32r = mybir.dt.float32r
    bf16 = mybir.dt.bfloat16
    nsb = len(widths)
    col0s = np.concatenate([[0], np.cumsum(widths)]).astype(int)
    qpad = int(col0s[-1])
    wmax = int(max(widths))
    nb_tot = blocks[-1]
    ns_tot = sum(nslots)

    order = _sb_order(nsb)
    tail_ma = _tail_maskadd_sbs(order)
    chunks, k_off, q_off, qkw, _ = _qk_layout(widths, blocks, nslots, c0bs)

    # tail mask-add tiles: [ident | per-slot [128, w-c0b] tiles]
    m_off = {}
    mw = KB
    for sb in order:
        if sb not in tail_ma:
            continue
        w = int(widths[sb])
        ci = sum(nslots[:sb])
        for s in range(nslots[sb]):
            m_off[(sb, blocks[sb] - 1 - s)] = mw
            mw += w - c0bs[ci + s]

    nc = bacc.Bacc(None, target_bir_lowering=False, debug=False)
    qk_d = nc.declare_dram_parameter("qk", [D, qkw], f32r, isOutput=False)
    vp_d = nc.declare_dram_parameter("vp", [KB, nb_tot * 65], f32r,
                                     isOutput=False)
    iot_d = nc.declare_dram_parameter("iot", [KB, wmax + max(ns_tot, 1)],
                                      f32, isOutput=False)
    mt_d = nc.declare_dram_parameter("mt", [KB, mw], bf16, isOutput=False)
    o_d = nc.declare_dram_parameter("o", [65, qpad], f32, isOutput=True)

    with tile.TileContext(nc) as tc:
        with (
            tc.tile_pool(name="const", bufs=1) as cpool,
            tc.tile_pool(name="pt", bufs=8) as ppool,
            tc.tile_pool(name="ob", bufs=5) as obpool,
            tc.tile_pool(name="ps", bufs=3, space=bass.MemorySpace.PSUM) as spool,
            tc.tile_pool(name="po", bufs=2, space=bass.MemorySpace.PSUM) as opool,
        ):
            qk = cpool.tile([D, qkw], f32r)
            vp = cpool.tile([KB, nb_tot * 65], f32r)
            iot = cpool.tile([KB, wmax + max(ns_tot, 1)], f32)
            mt = cpool.tile([KB, mw], bf16)
            ident = mt[:, 0:KB]

            # PE p-state warm-up: dummy matmuls on a zeroed tile keep the
            # tensor engine busy from t~0 so real matmuls hit full clock.
            wz = cpool.tile([D, 512], bf16, name="warm")
            nc.vector.memset(wz[:], 0.0)
            wps = spool.tile([KB, 512], f32, name="warmps", tag="s")
            for _ in range(N_WARMUP):
                nc.tensor.matmul(wps[:, 0:384], wz[:, 0:KB],
                                 wz[:, 0:384], start=True, stop=True)

            # Input DMAs: need-ordered contiguous qk chunks via SP->HWDGE
            # (chunk 0 carries everything the first superblock touches);
            # iot/vp/mask tiles via Pool->SWDGE so issue overheads overlap.
            vh = blocks[order[0]] * 65
            lo, hi = chunks[0]
            nc.sync.dma_start(qk[:, lo:hi], qk_d[:, lo:hi])
            nc.gpsimd.dma_start(iot[:], iot_d[:])
            nc.gpsimd.dma_start(vp[:, 0:vh], vp_d[:, 0:vh])
            for lo, hi in chunks[1:3]:
                nc.sync.dma_start(qk[:, lo:hi], qk_d[:, lo:hi])
            if vh < nb_tot * 65:
                nc.gpsimd.dma_start(vp[:, vh:], vp_d[:, vh:])
            for lo, hi in chunks[3:]:
                nc.sync.dma_start(qk[:, lo:hi], qk_d[:, lo:hi])
            nc.gpsimd.dma_start(mt[:], mt_d[:])

            # slot -> column in th, per (sb, slot); c0b per (sb, kb)
            th_col = {}
            c0b_of = {}
            c = 0
            for sb in range(nsb):
                for s in range(nslots[sb]):
                    th_col[(sb, s)] = c
                    c0b_of[(sb, blocks[sb] - 1 - s)] = c0bs[c]
                    c += 1

            items = []
            for sb in order:
                nkb = blocks[sb]
                w_ = int(widths[sb])
                grp = GRP if (w_ == 512 or 2 * w_ <= 512) else 1
                for g in range(-(-nkb // grp)):
                    gkbs = list(range(g * grp, min((g + 1) * grp, nkb)))
                    items.append((sb, g, nkb, gkbs))

            o_of = {}
            pending = None
            mask_n = 0

            def emit_pv(sb, gkbs, nkb, p):
                w = int(widths[sb])
                for j, kb in enumerate(gkbs):
                    cb = c0b_of.get((sb, kb), 0)
                    nc.tensor.matmul(
                        o_of[sb][:, cb:w],
                        vp[:, kb * 65:(kb + 1) * 65],
                        p[:, j * w + cb:(j + 1) * w],
                        start=(kb == 0), stop=(kb == nkb - 1),
                    )
                if gkbs[-1] == nkb - 1:
                    c0 = int(col0s[sb])
                    ob = obpool.tile([65, w], f32, name=f"ob{sb}", tag="ob")
                    if sb == order[-1]:
                        # last sb: Activation is idle by then and DVE is
                        # the tail serializer — drain there instead
                        nc.scalar.copy(ob[:], o_of[sb][:])
                    else:
                        nc.vector.tensor_copy(ob[:], o_of[sb][:])
                    if len(order) >= 4 and sb == order[-2]:
                        # Act is idle after its last exp; issuing here keeps
                        # SP free for the final sb's out-DMA
                        nc.scalar.dma_start(o_d[:, c0:c0 + w], ob[:])
                    else:
                        nc.sync.dma_start(o_d[:, c0:c0 + w], ob[:])
                    del o_of[sb]

            for sb, g, nkb, gkbs in items:
                w = int(widths[sb])
                if g == 0:
                    o_of[sb] = opool.tile([65, w], f32,
                                          name=f"oacc{sb}", tag="o_acc")
                qo = q_off[sb]
                gw = len(gkbs) * w
                s = spool.tile([KB, gw], f32)
                regs = []
                for j, kb in enumerate(gkbs):
                    cb = c0b_of.get((sb, kb), 0)
                    ma = (sb, kb) in m_off
                    nc.tensor.matmul(
                        s[:, j * w + cb:(j + 1) * w],
                        qk[:, k_off[kb]:k_off[kb] + KB],
                        qk[:, qo + cb:qo + w],
                        start=True, stop=not ma,
                    )
                    if ma:
                        # tail sbs: causal mask folded into the score psum
                        mo = m_off[(sb, kb)]
                        nc.tensor.matmul(
                            s[:, j * w + cb:(j + 1) * w],
                            ident,
                            mt[:, mo:mo + w - cb],
                            start=False, stop=True,
                        )
                    if regs and regs[-1][1] == j * w + cb:
                        regs[-1] = (regs[-1][0], (j + 1) * w)
                    else:
                        regs.append((j * w + cb, (j + 1) * w))
                if pending is not None:
                    emit_pv(*pending)
                p = ppool.tile([KB, gw], f32r)
                for a, b in regs:
                    nc.scalar.activation(
                        p[:, a:b], s[:, a:b],
                        mybir.ActivationFunctionType.Exp,
                    )
                # causal threshold masks on boundary blocks (non-tail sbs):
                # p *= (col_iota >= tc - col0)
                if sb not in tail_ma:
                    for sl in range(nslots[sb]):
                        kb = nkb - 1 - sl
                        if kb in gkbs:
                            j = kb - gkbs[0]
                            cb = c0b_of.get((sb, kb), 0)
                            tcc = wmax + th_col[(sb, sl)]
                            nc.vector.scalar_tensor_tensor(
                                p[:, j * w + cb:(j + 1) * w],
                                iot[:, cb:w],
                                iot[:, tcc:tcc + 1],
                                p[:, j * w + cb:(j + 1) * w],
                                op0=mybir.AluOpType.is_ge,
                                op1=mybir.AluOpType.mult,
                            )
                pending = (sb, gkbs, nkb, p)
            emit_pv(*pending)

    nc.compile()
    return nc


def _widths_for(qpad):
    """Ragged (<512) superblocks lead so their per-block (GRP=1) psum
    tiles stay inside one bank; each width is >=256 for full-rate f32r."""
    rem = qpad % 512
    rag = {0: (), 128: (256, 384), 256: (256,), 384: (128, 256)}[rem]
    return rag + (512,) * ((qpad - sum(rag)) // 512)


def _plan(qm, vm):
    """Uniform (widths, blocks, nslots) + per-batch/core live sets."""
    qlives = [np.flatnonzero(qm[b]) for b in range(B)]
    klives = []
    for c in range(NCORES):
        b, par = c // 2, c % 2
        klives.append(np.flatnonzero(vm[b])[par::2])

    nqmax = max(max(q.size for q in qlives), 2)
    qpad = max(256, -(-nqmax // KB) * KB)
    widths = _widths_for(qpad)
    nsb = len(widths)
    col0s = np.concatenate([[0], np.cumsum(widths)]).astype(int)

    blocks = []
    for sb in range(nsb):
        c1 = int(col0s[sb + 1])
        nb = 1
        for c in range(NCORES):
            qlive = qlives[c // 2]
            ncols = min(c1, qlive.size)
            if ncols == 0:
                continue
            qmax = qlive[ncols - 1]
            cnt = int(np.searchsorted(klives[c], qmax, side="right"))
            nb = max(nb, -(-cnt // KB))
        blocks.append(nb)
    for sb in range(1, nsb):
        blocks[sb] = max(blocks[sb], blocks[sb - 1])

    # tc per core: first compact column whose q_orig >= k_orig
    tcs = []
    for c in range(NCORES):
        qlive = qlives[c // 2]
        tcs.append(np.searchsorted(qlive, klives[c], side="left"))

    nslots = []
    for sb in range(nsb):
        c0 = int(col0s[sb])
        smax = 0
        for c in range(NCORES):
            live = klives[c]
            tc = tcs[c]
            n = 0
            for j in range(blocks[sb] - 1, -1, -1):
                lo, hi = j * KB, min((j + 1) * KB, live.size)
                if lo >= hi or tc[lo:hi].max() > c0:
                    n += 1          # padding or real boundary -> mask it
                else:
                    break
            smax = max(smax, n)
        nslots.append(min(smax, blocks[sb]))

    # first useful column per slotted block: all earlier columns are fully
    # masked, so score/exp/mask/pv can start there (min over cores; >=256
    # cols kept for full-rate f32r; kb==0 stays full for psum start=True).
    # A block that starts its own exp region subranges for free; one that
    # would split a merged exp instr must save >=224 cols (185ns instr
    # overhead) to pay for it.
    c0bs = []
    for sb in range(nsb):
        c0 = int(col0s[sb])
        w = int(widths[sb])
        grp = GRP if (w == 512 or 2 * w <= 512) else 1
        for s in range(nslots[sb]):
            kb = blocks[sb] - 1 - s
            if kb == 0:
                c0bs.append(0)
                continue
            raw = w - 64
            for c in range(NCORES):
                lo = kb * KB
                hi = min((kb + 1) * KB, klives[c].size)
                if lo < hi:
                    raw = min(raw, int(tcs[c][lo:hi].min()) - c0)
            # a block alone in its exp group can shrink below 256 cols:
            # the f32r 4x matmul penalty lands on PE (slack) while the
            # saturated Activation stream shrinks 1:1
            solo = grp == 1 or (kb == blocks[sb] - 1 and
                                blocks[sb] % grp == 1)
            cap = w - 256
            thr = 32 if (solo or kb % grp == 0) else 224
            cb = max(0, min(raw, cap)) // 32 * 32  # ISA-aligned widths
            c0bs.append(cb if cb >= thr else 0)
    return widths, blocks, nslots, tuple(c0bs), qlives, klives, tcs


def _get_nc(sig):
    key = (tuple(sig[0]), tuple(sig[1]), tuple(sig[2]), tuple(sig[3]))
    if key not in _compiled:
        _compiled[key] = _build_nc(*key)
    return _compiled[key]


def _host_inputs(query, value, keys, q_mask, v_mask, scale):
    scale = np.float32(scale)
    q = np.asarray(query, np.float32)
    v = np.asarray(value, np.float32)
    k = np.asarray(keys, np.float32)
    qm = np.asarray(q_mask).astype(bool)
    vm = np.asarray(v_mask).astype(bool)

    widths, blocks, nslots, c0bs, qlives, klives, tcs = _plan(qm, vm)
    sig = (widths, blocks, nslots, c0bs)
    nsb = len(widths)
    col0s = np.concatenate([[0], np.cumsum(widths)]).astype(int)
    qpad = int(col0s[-1])
    wmax = int(max(widths))
    nb_tot = blocks[-1]
    npad = nb_tot * KB
    ns_tot = sum(nslots)
    order = _sb_order(nsb)
    tail_ma = _tail_maskadd_sbs(order)
    _, k_off, q_off, qkw, pieces = _qk_layout(widths, blocks, nslots, c0bs)

    m_packs = []
    mw = KB
    for sb in order:
        if sb not in tail_ma:
            continue
        ci = sum(nslots[:sb])
        for s in range(nslots[sb]):
            m_packs.append((sb, blocks[sb] - 1 - s, c0bs[ci + s], mw))
            mw += int(widths[sb]) - c0bs[ci + s]

    iota = np.broadcast_to(np.arange(wmax, dtype=np.float32),
                           (KB, wmax))
    in_maps = []
    for c in range(NCORES):
        b = c // 2
        qlive = qlives[b]
        live = klives[c]
        tc = tcs[c]
        nl = live.size

        qt = np.zeros((D, qpad), np.float32)
        qt[:, :qlive.size] = q[b][qlive].T * scale
        kc = np.zeros((npad, D), np.float32)
        kc[:nl] = k[b][live]
        vc = np.zeros((npad, 65), np.float32)
        vc[:nl, :64] = v[b][live]
        vc[:nl, 64] = 1.0
        kt = np.ascontiguousarray(kc.T)
        vp = np.ascontiguousarray(
            vc.reshape(nb_tot, KB, 65).transpose(1, 0, 2).reshape(KB, -1))

        qk = np.empty((D, qkw), np.float32)
        off = 0
        for kind, lo, hi in pieces:
            src = kt[:, lo:hi] if kind == "k" else qt[:, lo:hi]
            qk[:, off:off + hi - lo] = src
            off += hi - lo

        tc_pad = np.full(npad, PAD_TH, np.float32)
        tc_pad[:nl] = tc
        th = np.zeros((KB, max(ns_tot, 1)), np.float32)
        col = 0
        for sb in range(nsb):
            for s in range(nslots[sb]):
                kb = blocks[sb] - 1 - s
                tv = tc_pad[kb * KB:(kb + 1) * KB].copy()
                real = tv < PAD_TH
                tv[real] -= col0s[sb]
                th[:, col] = tv
                col += 1

        iot = np.concatenate([iota, th], axis=1)
        mt = np.empty((KB, mw), np.float32)
        mt[:, 0:KB] = np.eye(KB, dtype=np.float32)
        for sb, kb, cb, moff in m_packs:
            w = int(widths[sb])
            thr = tc_pad[kb * KB:(kb + 1) * KB, None] - col0s[sb]
            cols = np.arange(cb, w)[None, :]
            mt[:, moff:moff + w - cb] = np.where(cols >= thr, 0.0,
                                                -np.float32(NEG_BIG))
        in_maps.append({"qk": qk, "vp": vp,
                        "iot": np.ascontiguousarray(iot),
                        "mt": _to_bf16(mt)})
    return in_maps, sig, qlives


def _to_bf16(a):
    import ml_dtypes

    return np.ascontiguousarray(a.astype(ml_dtypes.bfloat16))


def _host_gather(results, qlives, query, value, keys, q_mask, v_mask,
                 scale):
    q = np.asarray(query, np.float32)
    v = np.asarray(value, np.float32)
    k = np.asarray(keys, np.float32)
    qm = np.asarray(q_mask).astype(bool)
    vm = np.asarray(v_mask).astype(bool)
    scale = np.float32(scale)

    out = np.zeros((B, T, D), np.float32)
    for b in range(B):
        oT = results[2 * b]["o"] + results[2 * b + 1]["o"]
        qlive = qlives[b]
        nq = qlive.size
        l = oT[64, :nq]
        vals = (oT[:64, :nq] / np.where(l > 0, l, 1.0)).T
        out[b, qlive] = vals
        nz = np.flatnonzero(vm[b])
        first = nz[0] if nz.size else T
        if first > 0:
            rows = np.arange(first)
            s = ((q[b, rows] @ k[b].T) * scale).astype(np.float32)
            s = s - np.float32(NEG_BIG)
            s = s.astype(np.float64)
            s -= s.max(axis=1, keepdims=True)
            p = np.exp(s)
            p /= p.sum(axis=1, keepdims=True)
            out[b, rows] = p @ v[b].astype(np.float64)
    out = np.where(qm[..., None], out, np.float32(0.0))
    return out


def kernel(**inputs):
    from concourse.bass_utils import run_bass_kernel_spmd

    in_maps, sig, qlives = _host_inputs(**inputs)
    nc = _get_nc(sig)
    res = run_bass_kernel_spmd(nc, in_maps, list(range(NCORES))).results
    return _host_gather(res, qlives, **inputs)


# revision 88
# speedup vs baseline: 1.0022x; 1.0022x over previous
"""v5: query+key compacted sparse causal attention.

Sharding: 8 cores = 4 batches x 2 key-parity shards (key blocks of 128
interleaved across the core pair); host sums the two partial outputs.

Work reduction: keys dead under v_mask and queries dead under q_mask are
both compacted away on host (order-preserving), roughly quartering the
dense work.  Causal masking survives compaction because the
column->original-row map is monotone: for key row r, mask(col c) =
(c >= tc[r]) with tc[r] = searchsorted(q_orig, k_orig[r]) host data.
Boundary blocks also skip their fully-masked leading columns (c0b
subranges, ISA-aligned to 32) in the score/exp/mask/pv ops.

Schedule (per core, Activation-saturated ~12.5us of a ~21us kernel):
- superblock order (1, 2, big..., 0): small sbs warm the pipeline, the
  smallest drains last so the serial pv->copy->DMA tail is short;
- kt/qt packed into one dram param in need order so the first compute
  chunk is a single DMA; iot/vp/mask tiles ride Pool->SWDGE in parallel
  with the SP->HWDGE chunks;
- dummy bf16 matmuls warm the PE p-state before real work arrives
  (memset on DVE so Pool can start descriptor-gen immediately);
- mid sbs mask via one DVE scalar_tensor_tensor per boundary block
  (p *= iota >= tc - col0); the last two sbs instead fold the mask into
  the score psum with an identity-stationary matmul adding -1e9 tiles
  (PE has slack there, DVE would serialize the drain tail);
- output drains: DVE copies mid-kernel, Activation for the last sb
  (idle after its final exp).

Softmax runs unshifted (exp, no max-sub, fp32 psum accumulate) with the
denominator as a 65th V' column, divided out on host; padding keys carry
V'=0 and padding query columns are dropped by the host scatter.  Rows
before the first live key fall back to a host fp64 softmax.
"""

import sys

import numpy as np

try:
    import concourse  # noqa: F401
except ImportError:  # pragma: no cover
    sys.path.insert(0, "/opt/trn_rl_repo")

B, T, D = 4, 4096, 64
NCORES = 8
KB = 128
GRP = 2
N_WARMUP = 5
NEG_BIG = 1e9
PAD_TH = 1.0e9  # th sentinel for padding key rows (masks them to 0)

_compiled = {}


def _sb_order(nsb):
    """Small ragged sbs first (cheap pipeline warm-up), big ones in the
    middle, smallest (sb0) last so the serial drain tail is short."""
    if nsb == 1:
        return (0,)
    return tuple(range(1, 3)[:nsb - 1]) + tuple(range(nsb - 1, 2, -1)) + (0,)


def _tail_maskadd_sbs(order):
    """Tail sbs whose causal masks fold into the score psum via an
    identity-stationary matmul (PE) instead of a DVE multiply — takes the
    serial mask stage out of the drain tail.  Only safe for late sbs
    whose mask tiles have plenty of time to arrive."""
    return set(order[-2:]) if len(order) >= 4 else set()


def _qk_layout(widths, blocks, nslots, c0bs):
    """Need-ordered packing of kt|qt into one dram param so each input
    chunk is ONE contiguous DMA.  Returns (chunks, k_off, q_off, total)
    where chunks is a list of (start, end) col ranges in emission order,
    k_off[kb] is the param col of key block kb, q_off[sb] of sb's col0."""
    nsb = len(widths)
    order = _sb_order(nsb)
    col0s = np.concatenate([[0], np.cumsum(widths)]).astype(int)
    qpad = int(col0s[-1])
    ktw = blocks[-1] * KB
    c_a = int(col0s[order[0]])
    c_m = int(col0s[order[0] + 1])
    c_b = int(col0s[order[1] + 1]) if nsb > 1 else qpad
    kh0 = blocks[order[0]] * KB
    kh = blocks[order[min(1, nsb - 1)]] * KB

    pieces = [("k", 0, kh0), ("q", c_a, c_m)]
    if kh0 < kh:
        pieces.append(("k", kh0, kh))
    if c_m < c_b:
        pieces.append(("q", c_m, c_b))
    if c_b < qpad:
        pieces.append(("q", c_b, qpad))
    if kh < ktw:
        pieces.append(("k", kh, ktw))
    if c_a > 0:
        pieces.append(("q", 0, c_a))
    # chunk boundaries: first two pieces are chunk 0 (the critical first
    # transfer); after that one chunk per piece
    k_off = {}
    q_off = {}
    bounds = [0]
    off = 0
    for i, (kind, lo, hi) in enumerate(pieces):
        if kind == "k":
            for kb in range(lo // KB, hi // KB):
                k_off[kb] = off + (kb * KB - lo)
        else:
            for sb in range(nsb):
                if lo <= col0s[sb] and col0s[sb + 1] <= hi:
                    q_off[sb] = off + (int(col0s[sb]) - lo)
        off += hi - lo
        if i != 0:
            bounds.append(off)
    chunks = list(zip(bounds, bounds[1:]))
    return chunks, k_off, q_off, off, pieces


def _build_nc(widths, blocks, nslots, c0bs):
    import concourse.bass as bass
    import concourse.mybir as mybir
    import concourse.tile as tile
    from concourse import bacc

    f32 = mybir.dt.float32
    f32r = mybir.dt.float32r
    bf16 = mybir.dt.bfloat16
    nsb = len(widths)
    col0s = np.concatenate([[0], np.cumsum(widths)]).astype(int)
    qpad = int(col0s[-1])
    wmax = int(max(widths))
    nb_tot = blocks[-1]
    ns_tot = sum(nslots)

    order = _sb_order(nsb)
    tail_ma = _tail_maskadd_sbs(order)
    chunks, k_off, q_off, qkw, _ = _qk_layout(widths, blocks, nslots, c0bs)

    # tail mask-add tiles: [ident | per-slot [128, w-c0b] tiles]
    m_off = {}
    mw = KB
    for sb in order:
        if sb not in tail_ma:
            continue
        w = int(widths[sb])
        ci = sum(nslots[:sb])
        for s in range(nslots[sb]):
            m_off[(sb, blocks[sb] - 1 - s)] = mw
            mw += w - c0bs[ci + s]

    nc = bacc.Bacc(None, target_bir_lowering=False, debug=False)
    qk_d = nc.declare_dram_parameter("qk", [D, qkw], f32r, isOutput=False)
    vp_d = nc.declare_dram_parameter("vp", [KB, nb_tot * 65], f32r,
                                     isOutput=False)
    iot_d = nc.declare_dram_parameter("iot", [KB, wmax + max(ns_tot, 1)],
                                      f32, isOutput=False)
    mt_d = nc.declare_dram_parameter("mt", [KB, mw], bf16, isOutput=False)
    o_d = nc.declare_dram_parameter("o", [65, qpad], f32, isOutput=True)

    with tile.TileContext(nc) as tc:
        with (
            tc.tile_pool(name="const", bufs=1) as cpool,
            tc.tile_pool(name="pt", bufs=8) as ppool,
            tc.tile_pool(name="ob", bufs=5) as obpool,
            tc.tile_pool(name="ps", bufs=3, space=bass.MemorySpace.PSUM) as spool,
            tc.tile_pool(name="po", bufs=2, space=bass.MemorySpace.PSUM) as opool,
        ):
            qk = cpool.tile([D, qkw], f32r)
            vp = cpool.tile([KB, nb_tot * 65], f32r)
            iot = cpool.tile([KB, wmax + max(ns_tot, 1)], f32)
            mt = cpool.tile([KB, mw], bf16)
            ident = mt[:, 0:KB]

            # PE p-state warm-up: dummy matmuls on a zeroed tile keep the
            # tensor engine busy from t~0 so real matmuls hit full clock.
            wz = cpool.tile([D, 512], bf16, name="warm")
            nc.vector.memset(wz[:], 0.0)
            wps = spool.tile([KB, 512], f32, name="warmps", tag="s")
            for _ in range(N_WARMUP):
                nc.tensor.matmul(wps[:, 0:384], wz[:, 0:KB],
                                 wz[:, 0:384], start=True, stop=True)

            # Input DMAs: need-ordered contiguous qk chunks via SP->HWDGE
            # (chunk 0 carries everything the first superblock touches);
            # iot/vp/mask tiles via Pool->SWDGE so issue overheads overlap.
            vh = blocks[order[0]] * 65
            lo, hi = chunks[0]
            nc.sync.dma_start(qk[:, lo:hi], qk_d[:, lo:hi])
            nc.gpsimd.dma_start(iot[:], iot_d[:])
            nc.gpsimd.dma_start(vp[:, 0:vh], vp_d[:, 0:vh])
            for lo, hi in chunks[1:3]:
                nc.sync.dma_start(qk[:, lo:hi], qk_d[:, lo:hi])
            if vh < nb_tot * 65:
                nc.gpsimd.dma_start(vp[:, vh:], vp_d[:, vh:])
            for lo, hi in chunks[3:]:
                nc.sync.dma_start(qk[:, lo:hi], qk_d[:, lo:hi])
            nc.gpsimd.dma_start(mt[:], mt_d[:])

            # slot -> column in th, per (sb, slot); c0b per (sb, kb)
            th_col = {}
            c0b_of = {}
            c = 0
            for sb in range(nsb):
                for s in range(nslots[sb]):
                    th_col[(sb, s)] = c
                    c0b_of[(sb, blocks[sb] - 1 - s)] = c0bs[c]
                    c += 1

            items = []
            for sb in order:
                nkb = blocks[sb]
                w_ = int(widths[sb])
                grp = GRP if (w_ == 512 or 2 * w_ <= 512) else 1
                for g in range(-(-nkb // grp)):
                    gkbs = list(range(g * grp, min((g + 1) * grp, nkb)))
                    items.append((sb, g, nkb, gkbs))

            o_of = {}
            pending = None
            mask_n = 0

            def emit_pv(sb, gkbs, nkb, p):
                w = int(widths[sb])
                for j, kb in enumerate(gkbs):
                    cb = c0b_of.get((sb, kb), 0)
                    nc.tensor.matmul(
                        o_of[sb][:, cb:w],
                        vp[:, kb * 65:(kb + 1) * 65],
                        p[:, j * w + cb:(j + 1) * w],
                        start=(kb == 0), stop=(kb == nkb - 1),
                    )
                if gkbs[-1] == nkb - 1:
                    c0 = int(col0s[sb])
                    ob = obpool.tile([65, w], f32, name=f"ob{sb}", tag="ob")
                    if sb == order[-1]:
                        # last sb: Activation is idle by then and DVE is
                        # the tail serializer — drain there instead
                        nc.scalar.copy(ob[:], o_of[sb][:])
                    else:
                        nc.vector.tensor_copy(ob[:], o_of[sb][:])
                    if len(order) >= 4 and sb == order[-2]:
                        # Act is idle after its last exp; issuing here keeps
                        # SP free for the final sb's out-DMA
                        nc.scalar.dma_start(o_d[:, c0:c0 + w], ob[:])
                    else:
                        nc.sync.dma_start(o_d[:, c0:c0 + w], ob[:])
                    del o_of[sb]

            for sb, g, nkb, gkbs in items:
                w = int(widths[sb])
                if g == 0:
                    o_of[sb] = opool.tile([65, w], f32,
                                          name=f"oacc{sb}", tag="o_acc")
                qo = q_off[sb]
                gw = len(gkbs) * w
                s = spool.tile([KB, gw], f32)
                regs = []
                for j, kb in enumerate(gkbs):
                    cb = c0b_of.get((sb, kb), 0)
                    ma = (sb, kb) in m_off
                    nc.tensor.matmul(
                        s[:, j * w + cb:(j + 1) * w],
                        qk[:, k_off[kb]:k_off[kb] + KB],
                        qk[:, qo + cb:qo + w],
                        start=True, stop=not ma,
                    )
                    if ma:
                        # tail sbs: causal mask folded into the score psum
                        mo = m_off[(sb, kb)]
                        nc.tensor.matmul(
                            s[:, j * w + cb:(j + 1) * w],
                            ident,
                            mt[:, mo:mo + w - cb],
                            start=False, stop=True,
                        )
                    if regs and regs[-1][1] == j * w + cb:
                        regs[-1] = (regs[-1][0], (j + 1) * w)
                    else:
                        regs.append((j * w + cb, (j + 1) * w))
                if pending is not None:
                    emit_pv(*pending)
                p = ppool.tile([KB, gw], f32r)
                for a, b in regs:
                    nc.scalar.activation(
                        p[:, a:b], s[:, a:b],
                        mybir.ActivationFunctionType.Exp,
                    )
                # causal threshold masks on boundary blocks (non-tail sbs):
                # p *= (col_iota >= tc - col0)
                if sb not in tail_ma:
                    for sl in range(nslots[sb]):
                        kb = nkb - 1 - sl
                        if kb in gkbs:
                            j = kb - gkbs[0]
                            cb = c0b_of.get((sb, kb), 0)
                            tcc = wmax + th_col[(sb, sl)]
                            nc.vector.scalar_tensor_tensor(
                                p[:, j * w + cb:(j + 1) * w],
                                iot[:, cb:w],
                                iot[:, tcc:tcc + 1],
                                p[:, j * w + cb:(j + 1) * w],
                                op0=mybir.AluOpType.is_ge,
                                op1=mybir.AluOpType.mult,
                            )
                pending = (sb, gkbs, nkb, p)
            emit_pv(*pending)

    nc.compile()
    return nc


def _widths_for(qpad):
    """Ragged (<512) superblocks lead so their per-block (GRP=1) psum
    tiles stay inside one bank; each width is >=256 (full-rate f32r) and
    a multiple of 32 (matmul ISA alignment), with sb0 <=256 so it keeps
    the two-blocks-per-bank exp grouping."""
    rem = qpad % 512
    if rem == 0:
        rag = ()
    elif rem >= 256:
        rag = (rem,)
    else:
        rag = (256, rem + 256)
    return rag + (512,) * ((qpad - sum(rag)) // 512)


def _plan(qm, vm):
    """Uniform (widths, blocks, nslots) + per-batch/core live sets."""
    qlives = [np.flatnonzero(qm[b]) for b in range(B)]
    klives = []
    for c in range(NCORES):
        b, par = c // 2, c % 2
        klives.append(np.flatnonzero(vm[b])[par::2])

    nqmax = max(max(q.size for q in qlives), 2)
    qpad = max(256, -(-nqmax // 32) * 32)
    widths = _widths_for(qpad)
    nsb = len(widths)
    col0s = np.concatenate([[0], np.cumsum(widths)]).astype(int)

    blocks = []
    for sb in range(nsb):
        c1 = int(col0s[sb + 1])
        nb = 1
        for c in range(NCORES):
            qlive = qlives[c // 2]
            ncols = min(c1, qlive.size)
            if ncols == 0:
                continue
            qmax = qlive[ncols - 1]
            cnt = int(np.searchsorted(klives[c], qmax, side="right"))
            nb = max(nb, -(-cnt // KB))
        blocks.append(nb)
    for sb in range(1, nsb):
        blocks[sb] = max(blocks[sb], blocks[sb - 1])

    # tc per core: first compact column whose q_orig >= k_orig
    tcs = []
    for c in range(NCORES):
        qlive = qlives[c // 2]
        tcs.append(np.searchsorted(qlive, klives[c], side="left"))

    nslots = []
    for sb in range(nsb):
        c0 = int(col0s[sb])
        smax = 0
        for c in range(NCORES):
            live = klives[c]
            tc = tcs[c]
            n = 0
            for j in range(blocks[sb] - 1, -1, -1):
                lo, hi = j * KB, min((j + 1) * KB, live.size)
                if lo >= hi or tc[lo:hi].max() > c0:
                    n += 1          # padding or real boundary -> mask it
                else:
                    break
            smax = max(smax, n)
        nslots.append(min(smax, blocks[sb]))

    # first useful column per slotted block: all earlier columns are fully
    # masked, so score/exp/mask/pv can start there (min over cores; >=256
    # cols kept for full-rate f32r; kb==0 stays full for psum start=True).
    # A block that starts its own exp region subranges for free; one that
    # would split a merged exp instr must save >=224 cols (185ns instr
    # overhead) to pay for it.
    c0bs = []
    for sb in range(nsb):
        c0 = int(col0s[sb])
        w = int(widths[sb])
        grp = GRP if (w == 512 or 2 * w <= 512) else 1
        for s in range(nslots[sb]):
            kb = blocks[sb] - 1 - s
            if kb == 0:
                c0bs.append(0)
                continue
            raw = w - 64
            for c in range(NCORES):
                lo = kb * KB
                hi = min((kb + 1) * KB, klives[c].size)
                if lo < hi:
                    raw = min(raw, int(tcs[c][lo:hi].min()) - c0)
            # a block alone in its exp group can shrink below 256 cols:
            # the f32r 4x matmul penalty lands on PE (slack) while the
            # saturated Activation stream shrinks 1:1
            solo = grp == 1 or (kb == blocks[sb] - 1 and
                                blocks[sb] % grp == 1)
            cap = w - 256
            thr = 32 if (solo or kb % grp == 0) else 224
            cb = max(0, min(raw, cap)) // 32 * 32  # ISA-aligned widths
            c0bs.append(cb if cb >= thr else 0)
    return widths, blocks, nslots, tuple(c0bs), qlives, klives, tcs


def _get_nc(sig):
    key = (tuple(sig[0]), tuple(sig[1]), tuple(sig[2]), tuple(sig[3]))
    if key not in _compiled:
        _compiled[key] = _build_nc(*key)
    return _compiled[key]


def _host_inputs(query, value, keys, q_mask, v_mask, scale):
    scale = np.float32(scale)
    q = np.asarray(query, np.float32)
    v = np.asarray(value, np.float32)
    k = np.asarray(keys, np.float32)
    qm = np.asarray(q_mask).astype(bool)
    vm = np.asarray(v_mask).astype(bool)

    widths, blocks, nslots, c0bs, qlives, klives, tcs = _plan(qm, vm)
    sig = (widths, blocks, nslots, c0bs)
    nsb = len(widths)
    col0s = np.concatenate([[0], np.cumsum(widths)]).astype(int)
    qpad = int(col0s[-1])
    wmax = int(max(widths))
    nb_tot = blocks[-1]
    npad = nb_tot * KB
    ns_tot = sum(nslots)
    order = _sb_order(nsb)
    tail_ma = _tail_maskadd_sbs(order)
    _, k_off, q_off, qkw, pieces = _qk_layout(widths, blocks, nslots, c0bs)

    m_packs = []
    mw = KB
    for sb in order:
        if sb not in tail_ma:
            continue
        ci = sum(nslots[:sb])
        for s in range(nslots[sb]):
            m_packs.append((sb, blocks[sb] - 1 - s, c0bs[ci + s], mw))
            mw += int(widths[sb]) - c0bs[ci + s]

    iota = np.broadcast_to(np.arange(wmax, dtype=np.float32),
                           (KB, wmax))
    in_maps = []
    for c in range(NCORES):
        b = c // 2
        qlive = qlives[b]
        live = klives[c]
        tc = tcs[c]
        nl = live.size

        qt = np.zeros((D, qpad), np.float32)
        qt[:, :qlive.size] = q[b][qlive].T * scale
        kc = np.zeros((npad, D), np.float32)
        kc[:nl] = k[b][live]
        vc = np.zeros((npad, 65), np.float32)
        vc[:nl, :64] = v[b][live]
        vc[:nl, 64] = 1.0
        kt = np.ascontiguousarray(kc.T)
        vp = np.ascontiguousarray(
            vc.reshape(nb_tot, KB, 65).transpose(1, 0, 2).reshape(KB, -1))

        qk = np.empty((D, qkw), np.float32)
        off = 0
        for kind, lo, hi in pieces:
            src = kt[:, lo:hi] if kind == "k" else qt[:, lo:hi]
            qk[:, off:off + hi - lo] = src
            off += hi - lo

        tc_pad = np.full(npad, PAD_TH, np.float32)
        tc_pad[:nl] = tc
        th = np.zeros((KB, max(ns_tot, 1)), np.float32)
        col = 0
        for sb in range(nsb):
            for s in range(nslots[sb]):
                kb = blocks[sb] - 1 - s
                tv = tc_pad[kb * KB:(kb + 1) * KB].copy()
                real = tv < PAD_TH
                tv[real] -= col0s[sb]
                th[:, col] = tv
                col += 1

        iot = np.concatenate([iota, th], axis=1)
        mt = np.empty((KB, mw), np.float32)
        mt[:, 0:KB] = np.eye(KB, dtype=np.float32)
        for sb, kb, cb, moff in m_packs:
            w = int(widths[sb])
            thr = tc_pad[kb * KB:(kb + 1) * KB, None] - col0s[sb]
            cols = np.arange(cb, w)[None, :]
            mt[:, moff:moff + w - cb] = np.where(cols >= thr, 0.0,
                                                -np.float32(NEG_BIG))
        in_maps.append({"qk": qk, "vp": vp,
                        "iot": np.ascontiguousarray(iot),
                        "mt": _to_bf16(mt)})
    return in_maps, sig, qlives


def _to_bf16(a):
    import ml_dtypes

    return np.ascontiguousarray(a.astype(ml_dtypes.bfloat16))


def _host_gather(results, qlives, query, value, keys, q_mask, v_mask,
                 scale):
    q = np.asarray(query, np.float32)
    v = np.asarray(value, np.float32)
    k = np.asarray(keys, np.float32)
    qm = np.asarray(q_mask).astype(bool)
    vm = np.asarray(v_mask).astype(bool)
    scale = np.float32(scale)

    out = np.zeros((B, T, D), np.float32)
    for b in range(B):
        oT = results[2 * b]["o"] + results[2 * b + 1]["o"]
        qlive = qlives[b]
        nq = qlive.size
        l = oT[64, :nq]
        vals = (oT[:64, :nq] / np.where(l > 0, l, 1.0)).T
        out[b, qlive] = vals
        nz = np.flatnonzero(vm[b])
        first = nz[0] if nz.size else T
        if first > 0:
            rows = np.arange(first)
            s = ((q[b, rows] @ k[b].T) * scale).astype(np.float32)
            s = s - np.float32(NEG_BIG)
            s = s.astype(np.float64)
            s -= s.max(axis=1, keepdims=True)
            p = np.exp(s)
            p /= p.sum(axis=1, keepdims=True)
            out[b, rows] = p @ v[b].astype(np.float64)
    out = np.where(qm[..., None], out, np.float32(0.0))
    return out


def kernel(**inputs):
    from concourse.bass_utils import run_bass_kernel_spmd

    in_maps, sig, qlives = _host_inputs(**inputs)
    nc = _get_nc(sig)
    res = run_bass_kernel_spmd(nc, in_maps, list(range(NCORES))).results
    return _host_gather(res, qlives, **inputs)
